# revision 3
# baseline (speedup 1.0000x reference)
"""MiniGPT (12L, D=768, H=12, DFF=3072, V=50257, B=4, S=512) forward pass on
8 Trainium2 NeuronCores.

Sharding: data-parallel over batch across 4 core-pairs (pair p <-> sequence p),
plus a 2-way sequence split inside each pair. Core 2p owns query blocks
{0,3} of the sequence, core 2p+1 owns {1,2} (balanced causal work). Each core
runs attention for its own 256 queries and the FFN/residual stream for its own
256 tokens; K/V are computed (duplicated) for the full sequence from a
pairwise AllGather of the residual stream done once per layer. The AllGather
buffer is kept in pair-rank order so the device program is identical on all 8
cores (SPMD); all per-core variation (masks, token permutations) is input
data.

Numerics: all matmuls bf16 x bf16 with fp32 PSUM accumulation; layernorm,
softmax normalization and the residual stream are fp32. LayerNorm affine
(g,b) is folded into the adjacent weights on the host (exact here since
g=1,b=0). The causal mask is applied exactly like the reference: adding
-1e10 pre-scale in fp32 makes masked scores exactly -1.25e9, so exp
underflows to 0 and the q=0 row (fully masked in the reference) becomes
exactly uniform attention; we reproduce that with a rank-1 matmul fix-up
(cfix) that adds sum_k V[k] and 512 to the unnormalized row-0 output.

Weights are pre-swizzled on the host so each PSUM accumulation group streams
exactly one SBUF weight tile (keeps tile-pool slot lifetimes short).
"""

import numpy as np
import ml_dtypes
from einops import rearrange as ein

import concourse.bass as bass
import concourse.mybir as mybir
import concourse.tile as tile
from concourse import bacc
from concourse.bass_utils import run_bass_kernel_spmd
from concourse.masks import make_identity

# model dims (fixed for this problem)
L, H, D, HD, DFF, V, S, B = 12, 12, 768, 64, 3072, 50257, 512, 4
NEG = -1e10
P = 128
SH = S // 2          # tokens owned per core
NQT = SH // P        # 2 query tiles per core
NKB = S // P         # 4 key blocks
ND = D // P          # 6 contraction chunks of the model dim
NG = D // P          # 6 two-head groups (2*HD = 128)
NDFF = DFF // P      # 24
NH = 2               # 384-wide halves of a D-wide matmul output
DH = D // NH         # 384
NFG = 8              # fc2 contraction groups (3 dff-chunks each)
HDP1 = HD + 1        # V columns + ones column for row sums
VCH = 512            # vocab chunk for the LM head
NVT_FULL = (V + VCH - 1) // VCH  # 99

BF = mybir.dt.bfloat16
F32 = mybir.dt.float32
bf16 = ml_dtypes.bfloat16

RG_PAIRS = [[0, 1], [2, 3], [4, 5], [6, 7]]

_NC_CACHE = {}


def _emit_ln(nc, pst, x_ap, out_ap, eps_tile):
    """LayerNorm (standardize only) of [128, D] fp32 -> bf16 out."""
    st = pst.tile([P, 3, 6], F32, tag="lnst", name="lnst")
    xr = x_ap.rearrange("p (n f) -> p n f", f=256)
    for i in range(3):
        nc.vector.bn_stats(out=st[:, i, :], in_=xr[:, i, :])
    mv = pst.tile([P, 2], F32, tag="lnmv", name="lnmv")
    nc.vector.bn_aggr(out=mv[:], in_=st[:])
    rstd = pst.tile([P, 1], F32, tag="lnstd", name="lnstd")
    nc.scalar.activation(
        out=rstd[:], in_=mv[:, 1:2], func=mybir.ActivationFunctionType.Sqrt,
        bias=eps_tile[:], scale=1.0,
    )
    nc.vector.reciprocal(out=rstd[:], in_=rstd[:])
    nc.vector.tensor_scalar(
        out=out_ap, in0=x_ap, scalar1=mv[:, 0:1], scalar2=rstd[:],
        op0=mybir.AluOpType.subtract, op1=mybir.AluOpType.mult,
    )


def _emit_transpose(nc, pps_tp, src_ap, dst_ap, identity):
    """PE transpose of a [128,128] bf16 block, evict to SBUF."""
    pt = pps_tp.tile([P, P], BF, tag="tp", name="tp", space="PSUM")
    nc.tensor.transpose(pt[:], src_ap, identity[:])
    nc.any.tensor_copy(out=dst_ap, in_=pt[:])


def build_nc(n_layers=L, n_vt=NVT_FULL):
    nc = bacc.Bacc(None, target_bir_lowering=False, num_devices=8)

    # --- external I/O (per core) ---
    egkv_d = nc.dram_tensor("egkv", [S, D], F32, kind="ExternalInput")
    pkv_d = nc.dram_tensor("pkv", [S, D], F32, kind="ExternalInput")
    egown_d = nc.dram_tensor("egown", [SH, D], F32, kind="ExternalInput")
    pown_d = nc.dram_tensor("pown", [SH, D], F32, kind="ExternalInput")
    m1_d = nc.dram_tensor("m1", [NKB, P, SH], F32, kind="ExternalInput")
    cfix_d = nc.dram_tensor("cfix", [P, P], BF, kind="ExternalInput")
    wq_d = nc.dram_tensor("wq", [L, NG, P, ND, P], BF, kind="ExternalInput")
    wk_d = nc.dram_tensor("wk", [L, NG, P, ND, P], BF, kind="ExternalInput")
    wv_d = nc.dram_tensor("wv", [L, NH, P, ND, DH], BF, kind="ExternalInput")
    wo_d = nc.dram_tensor("wo", [L, NH, P, ND, DH], BF, kind="ExternalInput")
    w1_d = nc.dram_tensor("w1", [L, NDFF, P, ND, P], BF, kind="ExternalInput")
    w2_d = nc.dram_tensor("w2", [L, NFG, P, 3, D], BF, kind="ExternalInput")
    bq_d = nc.dram_tensor("bqp", [L, P, NG], F32, kind="ExternalInput")
    bk_d = nc.dram_tensor("bkp", [L, P, NG], F32, kind="ExternalInput")
    bv_d = nc.dram_tensor("bvrow", [L, 1, D], BF, kind="ExternalInput")
    bo_d = nc.dram_tensor("borow", [L, 1, D], BF, kind="ExternalInput")
    b1_d = nc.dram_tensor("b1p", [L, P, NDFF], F32, kind="ExternalInput")
    b2_d = nc.dram_tensor("b2row", [L, 1, D], BF, kind="ExternalInput")
    et_d = nc.dram_tensor("et", [D, V], BF, kind="ExternalInput")
    lb_d = nc.dram_tensor("lbias", [1, V], BF, kind="ExternalInput")
    logits_d = nc.dram_tensor("logits", [SH, V], F32, kind="ExternalOutput")

    et_ap = et_d[:].rearrange("(o p) v -> p o v", p=P)  # [128, 6, V]

    with tile.TileContext(nc) as tc:
        with (
            tc.tile_pool(name="pc", bufs=1) as pc,
            tc.tile_pool(name="p1", bufs=1) as p1,
            tc.tile_pool(name="p2", bufs=2) as p2,
            tc.tile_pool(name="p3", bufs=3) as p3,
            tc.tile_pool(name="pst", bufs=4) as pst,
            tc.tile_pool(name="pwa", bufs=3) as pwa,
            tc.tile_pool(name="pwb", bufs=2) as pwb,
            tc.tile_pool(name="pet", bufs=3) as pet,
            tc.tile_pool(name="pps_mm", bufs=4, space="PSUM") as pps_mm,
            tc.tile_pool(name="pps_st", bufs=1, space="PSUM") as pps_st,
            tc.tile_pool(name="pps_av", bufs=1, space="PSUM") as pps_av,
            tc.tile_pool(name="pps_tp", bufs=2, space="PSUM") as pps_tp,
            tc.tile_pool(name="pdram", bufs=1, space="DRAM") as pdram,
        ):
            # --- constants ---
            identity = pc.tile([P, P], BF, name="identity")
            make_identity(nc, identity[:])
            eps_tile = pc.tile([P, 1], F32, name="eps")
            nc.vector.memset(eps_tile[:], 1e-5)
            ones_row = pc.tile([1, P], BF, name="ones_row")
            nc.vector.memset(ones_row[:], 1.0)
            m1_sb = pc.tile([P, NKB, SH], F32, name="m1_sb")
            for kb in range(NKB):
                nc.sync.dma_start(m1_sb[:, kb, :], m1_d[kb, :, :])
            cfix_sb = pc.tile([P, P], BF, name="cfix_sb")
            nc.sync.dma_start(cfix_sb[:], cfix_d[:])

            # --- residual stream (persistent, own tokens) ---
            x_own = pc.tile([P, NQT, D], F32, name="x_own")
            for qt in range(NQT):
                ego = p2.tile([P, D], F32, tag="kvraw", name="ego")
                po = p2.tile([P, D], F32, tag="kvraw2", name="po")
                nc.sync.dma_start(ego[:], egown_d[qt * P:(qt + 1) * P, :])
                nc.sync.dma_start(po[:], pown_d[qt * P:(qt + 1) * P, :])
                nc.vector.tensor_add(out=x_own[:, qt, :], in0=ego[:], in1=po[:])

            cc_out_prev = None

            for l in range(n_layers):
                # ---------- own side: LN1 -> hq -> hqT, then Q ----------
                hq = p2.tile([P, NQT, D], BF, tag="hq", name="hq")
                for qt in range(NQT):
                    _emit_ln(nc, pst, x_own[:, qt, :], hq[:, qt, :], eps_tile)
                hqT = p2.tile([P, ND, SH], BF, tag="hqT", name="hqT")
                for qt in range(NQT):
                    for dc in range(ND):
                        _emit_transpose(
                            nc, pps_tp,
                            hq[:, qt, dc * P:(dc + 1) * P],
                            hqT[:, dc, qt * P:(qt + 1) * P],
                            identity,
                        )

                bq_sb = pst.tile([P, NG], F32, tag="bq", name="bq_sb")
                nc.sync.dma_start(bq_sb[:], bq_d[l, :, :])
                qall = p2.tile([P, NG, SH], BF, tag="qall", name="qall")
                for g in range(NG):
                    wt = pwa.tile([P, ND, P], BF, tag="wq", name="wq_t")
                    nc.sync.dma_start(wt[:], wq_d[l, g])
                    ps = pps_mm.tile([P, VCH], F32, tag="mm", name="ps_q", space="PSUM")
                    for dc in range(ND):
                        nc.tensor.matmul(
                            ps[:, :SH], lhsT=wt[:, dc, :],
                            rhs=hqT[:, dc, :], start=(dc == 0), stop=(dc == ND - 1),
                        )
                    nc.scalar.activation(
                        out=qall[:, g, :], in_=ps[:, :SH],
                        func=mybir.ActivationFunctionType.Identity,
                        bias=bq_sb[:, g:g + 1], scale=1.0,
                    )

                # ---------- kv side: load (AG output in rank order) ----------
                hkvT = p1.tile([P, ND, S], BF, tag="hkvT", name="hkvT")
                for sb in range(NKB):
                    raw = p2.tile([P, D], F32, tag="kvraw", name="kvraw")
                    if l == 0:
                        praw = p2.tile([P, D], F32, tag="kvraw2", name="pkvraw")
                        nc.sync.dma_start(raw[:], egkv_d[sb * P:(sb + 1) * P, :])
                        nc.sync.dma_start(praw[:], pkv_d[sb * P:(sb + 1) * P, :])
                        nc.vector.tensor_add(out=raw[:], in0=raw[:], in1=praw[:])
                    else:
                        nc.sync.dma_start(raw[:], cc_out_prev[sb * P:(sb + 1) * P, :])
                    hkv = p2.tile([P, D], BF, tag="hkv", name="hkv")
                    _emit_ln(nc, pst, raw[:], hkv[:], eps_tile)
                    for dc in range(ND):
                        _emit_transpose(
                            nc, pps_tp,
                            hkv[:, dc * P:(dc + 1) * P],
                            hkvT[:, dc, sb * P:(sb + 1) * P],
                            identity,
                        )

                # ---------- K (channel-partitioned, full sequence) ----------
                bk_sb = pst.tile([P, NG], F32, tag="bk", name="bk_sb")
                nc.sync.dma_start(bk_sb[:], bk_d[l, :, :])
                kall = p1.tile([P, NG, S], BF, tag="kall", name="kall")
                for g in range(NG):
                    wt = pwa.tile([P, ND, P], BF, tag="wk", name="wk_t")
                    nc.sync.dma_start(wt[:], wk_d[l, g])
                    ps = pps_mm.tile([P, VCH], F32, tag="mm", name="ps_k", space="PSUM")
                    for dc in range(ND):
                        nc.tensor.matmul(
                            ps[:, :S], lhsT=wt[:, dc, :],
                            rhs=hkvT[:, dc, :], start=(dc == 0), stop=(dc == ND - 1),
                        )
                    nc.scalar.activation(
                        out=kall[:, g, :], in_=ps[:, :S],
                        func=mybir.ActivationFunctionType.Identity,
                        bias=bk_sb[:, g:g + 1], scale=1.0,
                    )

                # ---------- V (token-partitioned, with ones column) ----------
                bv_sb = pst.tile([1, D], BF, tag="bv", name="bv_sb")
                nc.sync.dma_start(bv_sb[:], bv_d[l, :, :])
                vt = p1.tile([P, NKB, H, HDP1], BF, tag="vt", name="vt")
                nc.vector.memset(vt[:, :, :, HD:HDP1], 1.0)
                for nh in range(NH):
                    wt = pwb.tile([P, ND, DH], BF, tag="wv", name="wv_t")
                    nc.sync.dma_start(wt[:], wv_d[l, nh])
                    for sb in range(NKB):
                        ps = pps_mm.tile([P, VCH], F32, tag="mm", name="ps_v", space="PSUM")
                        for dc in range(ND):
                            nc.tensor.matmul(
                                ps[:, :DH],
                                lhsT=hkvT[:, dc, sb * P:(sb + 1) * P],
                                rhs=wt[:, dc, :], start=(dc == 0), stop=False,
                            )
                        nc.tensor.matmul(
                            ps[:, :DH], lhsT=ones_row[:],
                            rhs=bv_sb[:, nh * DH:(nh + 1) * DH],
                            start=False, stop=True,
                        )
                        nc.any.tensor_copy(
                            out=vt[:, sb, nh * 6:(nh + 1) * 6, 0:HD],
                            in_=ps[:, :DH].rearrange("p (h d) -> p h d", d=HD),
                        )
                vsum = p1.tile([P, H, HDP1], BF, tag="vsum", name="vsum")
                vtmp = p1.tile([P, H, HDP1], BF, tag="vtmp", name="vtmp")
                nc.vector.tensor_add(out=vtmp[:], in0=vt[:, 0], in1=vt[:, 1])
                nc.vector.tensor_add(out=vsum[:], in0=vt[:, 2], in1=vt[:, 3])
                nc.vector.tensor_add(out=vsum[:], in0=vsum[:], in1=vtmp[:])

                # ---------- attention ----------
                o_all = p1.tile([P, NQT, D], BF, tag="oall", name="oall")
                for h in range(H):
                    g, half = divmod(h, 2)
                    hp = slice(HD * half, HD * (half + 1))
                    at = p2.tile([P, NKB, SH], BF, tag="at", name="at")
                    for kb in range(NKB):
                        ps = pps_st.tile([P, SH], F32, tag="st", name="ps_st", space="PSUM")
                        nc.tensor.matmul(
                            ps[:], lhsT=kall[hp, g, kb * P:(kb + 1) * P],
                            rhs=qall[hp, g, :], start=True, stop=True,
                        )
                        nc.vector.tensor_add(out=ps[:], in0=ps[:], in1=m1_sb[:, kb, :])
                        nc.scalar.activation(
                            out=at[:, kb, :], in_=ps[:],
                            func=mybir.ActivationFunctionType.Exp, scale=0.125,
                        )
                    for qt in range(NQT):
                        po = pps_av.tile([P, HDP1], F32, tag="av", name="ps_av", space="PSUM")
                        for kb in range(NKB):
                            last = (kb == NKB - 1) and qt != 0
                            nc.tensor.matmul(
                                po[:], lhsT=at[:, kb, qt * P:(qt + 1) * P],
                                rhs=vt[:, kb, h, :], start=(kb == 0), stop=last,
                            )
                        if qt == 0:
                            nc.tensor.matmul(
                                po[:], lhsT=cfix_sb[:], rhs=vsum[:, h, :],
                                start=False, stop=True,
                            )
                        rec = pst.tile([P, 1], F32, tag="rec", name="rec")
                        nc.vector.reciprocal(out=rec[:], in_=po[:, HD:HDP1])
                        nc.vector.tensor_scalar_mul(
                            out=o_all[:, qt, h * HD:(h + 1) * HD],
                            in0=po[:, 0:HD], scalar1=rec[:],
                        )

                # ---------- Wo + residual ----------
                oT = p1.tile([P, ND, SH], BF, tag="oT", name="oT")
                for qt in range(NQT):
                    for dc in range(ND):
                        _emit_transpose(
                            nc, pps_tp,
                            o_all[:, qt, dc * P:(dc + 1) * P],
                            oT[:, dc, qt * P:(qt + 1) * P],
                            identity,
                        )
                bo_sb = pst.tile([1, D], BF, tag="bo", name="bo_sb")
                nc.sync.dma_start(bo_sb[:], bo_d[l, :, :])
                for nh in range(NH):
                    wt = pwb.tile([P, ND, DH], BF, tag="wo", name="wo_t")
                    nc.sync.dma_start(wt[:], wo_d[l, nh])
                    w_sl = slice(nh * DH, (nh + 1) * DH)
                    for qt in range(NQT):
                        ps = pps_mm.tile([P, VCH], F32, tag="mm", name="ps_wo", space="PSUM")
                        for dc in range(ND):
                            nc.tensor.matmul(
                                ps[:, :DH], lhsT=oT[:, dc, qt * P:(qt + 1) * P],
                                rhs=wt[:, dc, :], start=(dc == 0), stop=False,
                            )
                        nc.tensor.matmul(
                            ps[:, :DH], lhsT=ones_row[:], rhs=bo_sb[:, w_sl],
                            start=False, stop=True,
                        )
                        nc.vector.tensor_add(
                            out=x_own[:, qt, w_sl], in0=x_own[:, qt, w_sl],
                            in1=ps[:, :DH],
                        )

                # ---------- LN2 -> z2T ----------
                z2 = p1.tile([P, NQT, D], BF, tag="z2", name="z2")
                for qt in range(NQT):
                    _emit_ln(nc, pst, x_own[:, qt, :], z2[:, qt, :], eps_tile)
                z2T = p1.tile([P, ND, SH], BF, tag="z2T", name="z2T")
                for qt in range(NQT):
                    for dc in range(ND):
                        _emit_transpose(
                            nc, pps_tp,
                            z2[:, qt, dc * P:(dc + 1) * P],
                            z2T[:, dc, qt * P:(qt + 1) * P],
                            identity,
                        )

                # ---------- fc1 + gelu -> ffT ----------
                b1_sb = pst.tile([P, NDFF], F32, tag="b1", name="b1_sb")
                nc.sync.dma_start(b1_sb[:], b1_d[l, :, :])
                ffT = p1.tile([P, NDFF, SH], BF, tag="ffT", name="ffT")
                for db in range(NDFF):
                    wt = pwa.tile([P, ND, P], BF, tag="w1", name="w1_t")
                    nc.sync.dma_start(wt[:], w1_d[l, db])
                    ps = pps_mm.tile([P, VCH], F32, tag="mm", name="ps_f1", space="PSUM")
                    for dc in range(ND):
                        nc.tensor.matmul(
                            ps[:, :SH], lhsT=wt[:, dc, :],
                            rhs=z2T[:, dc, :], start=(dc == 0), stop=(dc == ND - 1),
                        )
                    nc.scalar.activation(
                        out=ffT[:, db, :], in_=ps[:, :SH],
                        func=mybir.ActivationFunctionType.Gelu,
                        bias=b1_sb[:, db:db + 1], scale=1.0,
                    )

                # ---------- fc2 + residual (long-lived psum groups) ----------
                b2_sb = pst.tile([1, D], BF, tag="b2", name="b2_sb")
                nc.sync.dma_start(b2_sb[:], b2_d[l, :, :])
                ps_f2 = [
                    pps_mm.tile([P, VCH], F32, tag="mm", name=f"ps_f2_{i}",
                                space="PSUM")
                    for i in range(NQT * NH)
                ]
                for fg in range(NFG):
                    wt = pwa.tile([P, 3, D], BF, tag="w2", name="w2_t")
                    nc.sync.dma_start(wt[:], w2_d[l, fg])
                    for qt in range(NQT):
                        for nh in range(NH):
                            ps = ps_f2[qt * NH + nh]
                            for c in range(3):
                                fc = fg * 3 + c
                                nc.tensor.matmul(
                                    ps[:, :DH],
                                    lhsT=ffT[:, fc, qt * P:(qt + 1) * P],
                                    rhs=wt[:, c, nh * DH:(nh + 1) * DH],
                                    start=(fg == 0 and c == 0), stop=False,
                                )
                for qt in range(NQT):
                    for nh in range(NH):
                        ps = ps_f2[qt * NH + nh]
                        w_sl = slice(nh * DH, (nh + 1) * DH)
                        nc.tensor.matmul(
                            ps[:, :DH], lhsT=ones_row[:], rhs=b2_sb[:, w_sl],
                            start=False, stop=True,
                        )
                        nc.vector.tensor_add(
                            out=x_own[:, qt, w_sl], in0=x_own[:, qt, w_sl],
                            in1=ps[:, :DH],
                        )

                # ---------- pairwise AllGather of the updated residual ----------
                if l < n_layers - 1:
                    cc_in = pdram.tile([SH, D], F32, name=f"ccin{l}")
                    cc_out = pdram.tile([S, D], F32, name=f"ccout{l}")
                    for qt in range(NQT):
                        nc.sync.dma_start(
                            cc_in[qt * P:(qt + 1) * P, :], x_own[:, qt, :]
                        )
                    nc.gpsimd.collective_compute(
                        "AllGather", mybir.AluOpType.bypass,
                        replica_groups=RG_PAIRS,
                        ins=[cc_in[:]], outs=[cc_out[:]],
                    )
                    cc_out_prev = cc_out

            # ---------- final LN + LM head ----------
            zf = p1.tile([P, NQT, D], BF, tag="z2", name="zf")
            for qt in range(NQT):
                _emit_ln(nc, pst, x_own[:, qt, :], zf[:, qt, :], eps_tile)
            zfT = p1.tile([P, ND, SH], BF, tag="z2T", name="zfT")
            for qt in range(NQT):
                for dc in range(ND):
                    _emit_transpose(
                        nc, pps_tp,
                        zf[:, qt, dc * P:(dc + 1) * P],
                        zfT[:, dc, qt * P:(qt + 1) * P],
                        identity,
                    )
            for vb in range(n_vt):
                vw = min(VCH, V - vb * VCH)
                et_t = pet.tile([P, ND, VCH], BF, tag="et", name="et_t")
                nc.sync.dma_start(
                    et_t[:, :, :vw], et_ap[:, :, vb * VCH:vb * VCH + vw]
                )
                lb_t = pst.tile([1, VCH], BF, tag="lb", name="lb_t")
                nc.sync.dma_start(lb_t[:, :vw], lb_d[:, vb * VCH:vb * VCH + vw])
                for qt in range(NQT):
                    ps = pps_mm.tile([P, VCH], F32, tag="mm", name="ps_lm", space="PSUM")
                    for dc in range(ND):
                        nc.tensor.matmul(
                            ps[:, :vw], lhsT=zfT[:, dc, qt * P:(qt + 1) * P],
                            rhs=et_t[:, dc, :vw], start=(dc == 0), stop=False,
                        )
                    nc.tensor.matmul(
                        ps[:, :vw], lhsT=ones_row[:], rhs=lb_t[:, :vw],
                        start=False, stop=True,
                    )
                    lo = p3.tile([P, VCH], F32, tag="lo", name="lo")
                    nc.any.tensor_copy(out=lo[:, :vw], in_=ps[:, :vw])
                    nc.sync.dma_start(
                        logits_d[qt * P:(qt + 1) * P, vb * VCH:vb * VCH + vw],
                        lo[:, :vw],
                    )

    nc.compile()
    return nc


# ---------------------------------------------------------------------------
# host side
# ---------------------------------------------------------------------------

def _core_token_order(parity):
    """Global token indices (within the sequence) owned by a core, and the
    pair-rank key order shared by both cores of the pair."""
    own_blocks = [0, 3] if parity == 0 else [1, 2]
    q_glob = np.concatenate([np.arange(b * P, (b + 1) * P) for b in own_blocks])
    k_blocks = [0, 3, 1, 2]  # even core's halves first (pair-rank order)
    k_glob = np.concatenate([np.arange(b * P, (b + 1) * P) for b in k_blocks])
    return q_glob, k_glob


def _prep_shared(E, Ppos, Wq, Wk, Wv, Wo, bo, g1, b1, g2, b2, W1, bff1, W2,
                 bff2, gf, bf_):
    """Fold layernorm affines into weights, pre-transpose + swizzle."""
    f32 = np.float32
    wq = np.empty((L, NG, P, ND, P), bf16)
    wk = np.empty((L, NG, P, ND, P), bf16)
    wv = np.empty((L, NH, P, ND, DH), bf16)
    wo = np.empty((L, NH, P, ND, DH), bf16)
    w1 = np.empty((L, NDFF, P, ND, P), bf16)
    w2 = np.empty((L, NFG, P, 3, D), bf16)
    bqp = np.empty((L, P, NG), f32)
    bkp = np.empty((L, P, NG), f32)
    bvrow = np.empty((L, 1, D), bf16)
    borow = np.empty((L, 1, D), bf16)
    b1p = np.empty((L, P, NDFF), f32)
    b2row = np.empty((L, 1, D), bf16)
    for l in range(L):
        wq_f = (Wq[l] * g1[l][None, None, :]).reshape(D, D).T  # [d, out]
        wk_f = (Wk[l] * g1[l][None, None, :]).reshape(D, D).T
        wv_f = (Wv[l] * g1[l][None, None, :]).reshape(D, D).T
        wq[l] = ein(wq_f.astype(bf16), "(dc p) (g f) -> g p dc f", p=P, f=P)
        wk[l] = ein(wk_f.astype(bf16), "(dc p) (g f) -> g p dc f", p=P, f=P)
        wv[l] = ein(wv_f.astype(bf16), "(dc p) (nh j) -> nh p dc j", p=P, j=DH)
        bq = Wq[l].reshape(D, D) @ b1[l]
        bk = Wk[l].reshape(D, D) @ b1[l]
        bv = Wv[l].reshape(D, D) @ b1[l]
        bqp[l] = bq.reshape(NG, P).T
        bkp[l] = bk.reshape(NG, P).T
        bvrow[l, 0] = bv.astype(bf16)
        wo[l] = ein(Wo[l].T.astype(bf16), "(dc p) (nh j) -> nh p dc j",
                    p=P, j=DH)
        borow[l, 0] = bo[l].astype(bf16)
        w1_f = (W1[l] * g2[l][None, :]).T  # [D, DFF]
        w1[l] = ein(w1_f.astype(bf16), "(dc p) (db f) -> db p dc f", p=P, f=P)
        b1u = W1[l] @ b2[l] + bff1[l]
        b1p[l] = b1u.reshape(NDFF, P).T
        w2[l] = ein(W2[l].T.astype(bf16), "(fg c p) e -> fg p c e", c=3, p=P)
        b2row[l, 0] = bff2[l].astype(bf16)
    sh = dict(wq=wq, wk=wk, wv=wv, wo=wo, w1=w1, w2=w2, bqp=bqp, bkp=bkp,
              bvrow=bvrow, borow=borow, b1p=b1p, b2row=b2row)
    sh["et"] = np.ascontiguousarray((E * gf[None, :]).T).astype(bf16)  # [D, V]
    sh["lbias"] = (E @ bf_).astype(f32)[None, :].astype(bf16)
    return sh


def _prep_core(tokens_seq, E, Ppos, parity):
    f32 = np.float32
    q_glob, k_glob = _core_token_order(parity)
    m1 = np.where(
        k_glob[:, None] < q_glob[None, :], f32(0.0), f32(NEG)
    ).astype(f32)  # [S(keys), SH(queries)]
    m1 = m1.reshape(NKB, P, SH)
    cfix = np.zeros((P, P), f32)
    hit = np.nonzero(q_glob[:P] == 0)[0]
    for j in hit:
        cfix[:, j] = 1.0
    toks = np.asarray(tokens_seq).astype(np.int64)
    return {
        "egkv": np.ascontiguousarray(E[toks[k_glob]]).astype(f32),
        "pkv": np.ascontiguousarray(Ppos[k_glob]).astype(f32),
        "egown": np.ascontiguousarray(E[toks[q_glob]]).astype(f32),
        "pown": np.ascontiguousarray(Ppos[q_glob]).astype(f32),
        "m1": np.ascontiguousarray(m1),
        "cfix": cfix.astype(bf16),
    }


def make_in_maps(tokens, E, Ppos, shared):
    in_maps = []
    for c in range(8):
        pair, parity = divmod(c, 2)
        core = _prep_core(tokens[pair], E, Ppos, parity)
        core.update(shared)
        in_maps.append(core)
    return in_maps


def assemble_output(results, dtype=np.float32):
    out = np.empty((B, S, V), dtype)
    for c in range(8):
        pair, parity = divmod(c, 2)
        q_glob, _ = _core_token_order(parity)
        out[pair, q_glob, :] = results[c]["logits"]
    return out


def get_nc(n_layers=L, n_vt=NVT_FULL):
    key = (n_layers, n_vt)
    if key not in _NC_CACHE:
        _NC_CACHE[key] = build_nc(n_layers, n_vt)
    return _NC_CACHE[key]


def kernel(tokens, E, P, Wq, Wk, Wv, Wo, bo, g1, b1, g2, b2, W1, bff1, W2,
           bff2, gf, bf, **_unused):
    tokens = np.asarray(tokens)
    args = [np.asarray(a, np.float32) for a in
            (E, P, Wq, Wk, Wv, Wo, bo, g1, b1, g2, b2, W1, bff1, W2, bff2,
             gf, bf)]
    (E, Ppos, Wq, Wk, Wv, Wo, bo, g1, b1, g2, b2, W1, bff1, W2, bff2,
     gf, bf_) = args
    shared = _prep_shared(E, Ppos, Wq, Wk, Wv, Wo, bo, g1, b1, g2, b2, W1,
                          bff1, W2, bff2, gf, bf_)
    in_maps = make_in_maps(tokens, E, Ppos, shared)
    nc = get_nc()
    res = run_bass_kernel_spmd(nc, in_maps, core_ids=list(range(8)))
    return assemble_output(res.results)
